# revision 28
# baseline (speedup 1.0000x reference)
"""Trainium2 Bass kernel for nn_CrossAttention (B=8, C=512, H=W=32, Lc=1024,
8 heads x 64 dim).

Sharding: data-parallel over batch B across the 8 NeuronCores (1 image/core,
no collectives). Feature-on-partitions layout; all matmuls contract over SBUF
partitions.

Optimizations over the v1 kernel (271.8us):
  - bf16 inputs (host-cast): 4MB instead of 8MB HBM per core.
  - Input DMAs spread across both HWDGE rings (sync + scalar) + SWDGE; no
    4-byte-element descriptor DMAs.
  - sim matmuls row-tiled: the two K=64 heads of a pair run concurrently in
    separate PE row-groups (col tiling is not supported by the compiler).
  - PV uses the ones-augmented vT (65 cols/head) so softmax denominators
    fall out of the PV matmul for free.
  - ACT does ONLY Exp (one table load). RMS rsqrt rows are computed in
    transposed [128, n] form via per-chunk stats matmuls (N=2), then a
    Quake-seed + 2x Newton rsqrt on DVE. Softmax reciprocal via
    nc.vector.reciprocal.
  - Per-token RMS factor rc of the context is applied for free as the
    per-partition ACT scale of the exp, and folded into vT at copy time.
  - exp runs over [128, 1024] PSUM pair-tiles (two heads per ACT op).
  - PE warmed up with dummy matmuls during the input-DMA window (HAM clock).
  - n-major stage C; stage D of n=0 hides under the ACT window of n=1.
  - software-pipelined emission: per-engine FIFOs never stall on bank WARs.
"""

import numpy as np
import ml_dtypes
from contextlib import ExitStack

import concourse.bass as bass
from concourse import bacc
import concourse.mybir as mybir
import concourse.tile as tile
from concourse.bass_utils import run_bass_kernel_spmd

F32 = mybir.dt.float32
F32R = mybir.dt.float32r
BF16 = mybir.dt.bfloat16
I32 = mybir.dt.int32
AF = mybir.ActivationFunctionType
OP = mybir.AluOpType

B, C, H, W = 8, 512, 32, 32
L = H * W  # 1024 query pixels
LC = 1024  # context tokens
HEADS, HD = 8, 64
VW = HD + 1  # 65: v columns + ones column (emits softmax denominator)
HID = HEADS * HD  # 512
EPS = 1e-6
NCORES = 8
CT = C // 128  # 4 c-tiles
JT = LC // 128  # 8 j-tiles

MAGIC = 0x5F3759DF


def build():
    nc = bacc.Bacc("TRN2", target_bir_lowering=False, debug=False,
                   num_devices=NCORES)

    x_d = nc.dram_tensor("x", [C, L], BF16, kind="ExternalInput")
    ct_d = nc.dram_tensor("ctxT", [C, LC], BF16, kind="ExternalInput")
    wq_d = nc.dram_tensor("wq", [C, HID], BF16, kind="ExternalInput")
    wk_d = nc.dram_tensor("wk", [C, HID], BF16, kind="ExternalInput")
    wv_d = nc.dram_tensor("wv", [C, HID], BF16, kind="ExternalInput")
    wo_d = nc.dram_tensor("wo", [HID, C], BF16, kind="ExternalInput")
    ones_d = nc.dram_tensor("ones", [128, 128], F32R, kind="ExternalInput")
    ident_d = nc.dram_tensor("ident", [128, 128], F32R, kind="ExternalInput")
    bog2_d = nc.dram_tensor("bog2T", [2, C], F32R, kind="ExternalInput")
    y_d = nc.dram_tensor("y_out", [C, L], F32, kind="ExternalOutput")

    with tile.TileContext(nc) as tc, ExitStack() as top:
        pc = top.enter_context(tc.tile_pool(name="main", bufs=1))
        psum = top.enter_context(tc.tile_pool(name="ps", bufs=1, space="PSUM"))

        # ---------------- input DMAs (spread across rings) ----------------
        ct_sb, x_sb = [], []
        for t in range(CT):
            ctt = pc.tile([128, LC], BF16, tag=f"ct{t}")
            nc.sync.dma_start(out=ctt, in_=ct_d[t * 128:(t + 1) * 128, :])
            ct_sb.append(ctt)
        for t in range(CT):
            xt = pc.tile([128, L], BF16, tag=f"x{t}")
            nc.scalar.dma_start(out=xt, in_=x_d[t * 128:(t + 1) * 128, :])
            x_sb.append(xt)
        wk_sb, wq_sb, wv_sb = [], [], []
        for name, lst, dram in (("wk", wk_sb, wk_d), ("wq", wq_sb, wq_d),
                                ("wv", wv_sb, wv_d)):
            for t in range(CT):
                wt = pc.tile([128, HID], BF16, tag=f"{name}{t}")
                nc.scalar.dma_start(out=wt, in_=dram[t * 128:(t + 1) * 128, :])
                lst.append(wt)
        ones_sb = pc.tile([128, 128], F32R, tag="ones")
        nc.gpsimd.dma_start(out=ones_sb, in_=ones_d[:, :])
        ident_sb = pc.tile([128, 128], F32R, tag="ident")
        nc.gpsimd.dma_start(out=ident_sb, in_=ident_d[:, :])
        bog2_sb = pc.tile([2, C], F32R, tag="bog2")
        nc.gpsimd.dma_start(out=bog2_sb, in_=bog2_d[:, :])
        wo_sb = []
        for t in range(CT):
            wt = pc.tile([128, C], BF16, tag=f"wo{t}")
            nc.gpsimd.dma_start(out=wt, in_=wo_d[t * 128:(t + 1) * 128, :])
            wo_sb.append(wt)

        # ---------------- PE warmup (runs during DMA wait) ----------------
        warm_sb = pc.tile([128, 512], F32, tag="warm")
        nc.vector.memset(warm_sb, 1.0)
        warm_ps = psum.tile([128, 512], F32, tag="spare", name="warmps",
                            bufs=2)
        for i in range(10):
            nc.tensor.matmul(out=warm_ps[:, :],
                             lhsT=warm_sb[:, 0:128].bitcast(F32R),
                             rhs=warm_sb[:, :].bitcast(F32R),
                             start=True, stop=True)
        warm_ex = pc.tile([1, 8], F32R, tag="warmex")
        nc.scalar.activation(out=warm_ex[:, :], in_=warm_sb[0:1, 0:8],
                             func=AF.Exp, bias=0.0, scale=0.0)

        # ---------------- squares + transposed stats -----------------------
        # ssq cols (pairs, col 2c used): 0:16 x-pixel chunks, 16:32 ctx
        ssq_ps = psum.tile([128, 512], F32, tag="sim", name="ssqps", bufs=2)
        sq_x, sq_c = [], []
        for t in range(CT):
            s = pc.tile([128, L], F32R, tag="sq", name=f"sqx{t}", bufs=8)
            nc.scalar.activation(out=s[:, :], in_=x_sb[t][:, :],
                                 func=AF.Square, bias=0.0, scale=1.0)
            sq_x.append(s)
        for t in range(CT):
            s = pc.tile([128, LC], F32R, tag="sq", name=f"sqc{t}", bufs=8)
            nc.vector.tensor_mul(s[:, :], ct_sb[t][:, :], ct_sb[t][:, :])
            sq_c.append(s)
        for c in range(8):
            for t in range(CT):
                nc.tensor.matmul(out=ssq_ps[:, 2 * c:2 * c + 2],
                                 lhsT=sq_x[t][:, c * 128:(c + 1) * 128],
                                 rhs=ones_sb[:, 0:2],
                                 start=(t == 0), stop=(t == CT - 1))
        for c in range(8):
            for t in range(CT):
                nc.tensor.matmul(out=ssq_ps[:, 16 + 2 * c:18 + 2 * c],
                                 lhsT=sq_c[t][:, c * 128:(c + 1) * 128],
                                 rhs=ones_sb[:, 0:2],
                                 start=(t == 0), stop=(t == CT - 1))

        # Quake rsqrt on DVE: dst = (src/nfeat + eps)^-0.5
        kmagic = pc.tile([128, 32], I32, tag="kmagic")
        nc.vector.memset(kmagic, MAGIC)

        def dve_rsqrt(dst, src_ps, ncols, nfeat, scratch_tag):
            m = pc.tile([128, ncols], F32, tag=f"{scratch_tag}m")
            nc.vector.tensor_scalar(out=m[:, :], in0=src_ps[:, 0:ncols],
                                    scalar1=1.0 / nfeat, scalar2=EPS,
                                    op0=OP.mult, op1=OP.add)
            m2 = pc.tile([128, ncols], F32, tag=f"{scratch_tag}m2")
            nc.vector.tensor_scalar(out=m2[:, :], in0=src_ps[:, 0:ncols],
                                    scalar1=0.5 / nfeat, scalar2=0.5 * EPS,
                                    op0=OP.mult, op1=OP.add)
            sh = pc.tile([128, ncols], I32, tag=f"{scratch_tag}sh")
            nc.vector.tensor_scalar(out=sh[:, :],
                                    in0=m[:, :].bitcast(I32),
                                    scalar1=1, scalar2=0,
                                    op0=OP.logical_shift_right,
                                    op1=OP.logical_shift_right)
            y0 = pc.tile([128, ncols], F32, tag=f"{scratch_tag}y0")
            nc.vector.scalar_tensor_tensor(
                out=y0[:, :].bitcast(I32), in0=kmagic[:, 0:ncols], scalar=0,
                in1=sh[:, :], op0=OP.add, op1=OP.subtract)
            # 2 Newton iters, negated form (signs cancel):
            # y' = (m2*y^2 - 1.5) * y
            t1 = pc.tile([128, ncols], F32, tag=f"{scratch_tag}t1")
            y1 = pc.tile([128, ncols], F32, tag=f"{scratch_tag}y1")
            nc.vector.tensor_mul(t1[:, :], y0[:, :], y0[:, :])
            nc.vector.tensor_mul(t1[:, :], t1[:, :], m2[:, :])
            nc.vector.scalar_tensor_tensor(
                out=y1[:, :], in0=t1[:, :], scalar=1.5, in1=y0[:, :],
                op0=OP.subtract, op1=OP.mult)
            nc.vector.tensor_mul(t1[:, :], y1[:, :], y1[:, :])
            nc.vector.tensor_mul(t1[:, :], t1[:, :], m2[:, :])
            nc.vector.scalar_tensor_tensor(
                out=dst[:, :], in0=t1[:, :], scalar=1.5, in1=y1[:, :],
                op0=OP.subtract, op1=OP.mult)

        # ---------------- projection machinery -----------------------------
        q_sb = [pc.tile([128, L], F32R, tag=f"q{m}", name=f"q{m}")
                for m in range(CT)]
        k_sb = [pc.tile([128, LC], F32R, tag=f"k{m}", name=f"k{m}")
                for m in range(CT)]
        vT_sb = []
        for j in range(JT):
            vt = pc.tile([128, HEADS * VW], F32R, tag=f"vT{j}", name=f"vT{j}")
            vh = vt[:, :].rearrange("p (h c) -> p h c", h=HEADS)
            nc.vector.memset(vh[:, :, HD:VW].bitcast(F32), 1.0)
            vT_sb.append(vt)
        ao_sb = [pc.tile([128, L], BF16, tag=f"ao{m}", name=f"ao{m}")
                 for m in range(CT)]
        rsq_xc = pc.tile([128, 32], F32, tag="rsqxc")
        bcx_sb = pc.tile([128, L], F32R, tag="bcx")

        def proj_q(m, n, ptag):
            ns = slice(n * 512, (n + 1) * 512)
            ps = psum.tile([128, 512], F32, tag=ptag, name=f"qp{m}{n}",
                           bufs=2)
            for t in range(CT):
                nc.tensor.matmul(out=ps[:, :],
                                 lhsT=wq_sb[t][:, m * 128:(m + 1) * 128],
                                 rhs=x_sb[t][:, ns],
                                 start=(t == 0), stop=(t == CT - 1))
            nc.vector.tensor_mul(q_sb[m][:, ns], ps[:, :],
                                 bcx_sb[:, ns].bitcast(F32))

        def proj_k(m, h, ptag):
            hs = slice(h * 512, (h + 1) * 512)
            ps = psum.tile([128, 512], F32, tag=ptag, name=f"kp{m}{h}",
                           bufs=2)
            for t in range(CT):
                nc.tensor.matmul(out=ps[:, :],
                                 lhsT=wk_sb[t][:, m * 128:(m + 1) * 128],
                                 rhs=ct_sb[t][:, hs],
                                 start=(t == 0), stop=(t == CT - 1))
            nc.vector.tensor_copy(k_sb[m][:, hs], ps[:, :])

        def proj_v(j, ptag):
            ps = psum.tile([128, HID], F32, tag=ptag, name=f"vp{j}",
                           bufs=2)
            for t in range(CT):
                nc.tensor.matmul(out=ps[:, :],
                                 lhsT=ct_sb[t][:, j * 128:(j + 1) * 128],
                                 rhs=wv_sb[t][:, :],
                                 start=(t == 0), stop=(t == CT - 1))
            vh = vT_sb[j][:, :].rearrange("p (h c) -> p h c", h=HEADS)
            # fold per-token rms factor rc into v
            nc.vector.tensor_scalar_mul(
                vh[:, :, 0:HD],
                ps[:, :].rearrange("p (h c) -> p h c", h=HEADS),
                rsq_xc[:, 16 + 2 * j:17 + 2 * j])

        # k projections first on the PE queue (only need ctx + wk DMAs)
        proj_k(0, 0, "spare")
        proj_k(0, 1, "ou")
        proj_k(1, 0, "spare")
        proj_k(1, 1, "ou")

        # rsq_xc cols (2c): 0:16 pixels rxT, 16:32 tokens rcT
        dve_rsqrt(rsq_xc, ssq_ps, 32, C, "rs")

        # bc_rx [128, L]: bc_rx[p, i] = rx[i] via diag trick
        diag_t = [pc.tile([128, 128], F32R, tag="diag", name=f"dg{c}", bufs=4)
                  for c in range(8)]
        bcx_ps = psum.tile([128, L], F32, tag="sim", name="bcxps", bufs=2)
        for c in range(8):
            nc.vector.tensor_scalar_mul(diag_t[c][:, :],
                                        ident_sb[:, :].bitcast(F32),
                                        rsq_xc[:, 2 * c:2 * c + 1])
            nc.tensor.matmul(out=bcx_ps[:, c * 128:(c + 1) * 128],
                             lhsT=ones_sb[:, :], rhs=diag_t[c][:, :],
                             start=True, stop=True)
        nc.vector.tensor_copy(bcx_sb[:, :], bcx_ps[:, :])

        # bog2 "transpose": [2, C] row layout -> [128, 2] per c-tile
        bo_sb, g2_sb = [], []
        for t in range(CT):
            bps = psum.tile([128, 512], F32, tag="ou", name=f"bog{t}", bufs=2)
            nc.tensor.matmul(out=bps[:, 0:2],
                             lhsT=bog2_sb[:, t * 128:(t + 1) * 128],
                             rhs=ident_sb[0:2, 0:2],
                             start=True, stop=True)
            bg = pc.tile([128, 2], F32, tag=f"bog2s{t}")
            nc.vector.tensor_copy(bg[:, :], bps[:, 0:2])
            bo_sb.append(bg[:, 0:1])
            g2_sb.append(bg[:, 1:2])

        # rest of the pre-attention projections
        proj_q(0, 0, "spare")
        proj_q(1, 0, "ou")
        proj_v(0, "spare")
        proj_v(1, "ou")
        proj_v(2, "spare")
        proj_v(3, "ou")
        proj_v(4, "spare")
        proj_v(5, "ou")

        # deferred projection work, drained into stage-C PE slack
        filler = []
        filler.append(lambda: proj_v(6, "spare"))
        filler.append(lambda: proj_v(7, "spare"))
        filler.append(lambda: proj_k(2, 0, "spare"))
        filler.append(lambda: proj_k(2, 1, "spare"))
        filler.append(lambda: proj_k(3, 0, "spare"))
        filler.append(lambda: proj_k(3, 1, "spare"))
        filler.append(lambda: proj_q(2, 0, "spare"))
        filler.append(lambda: proj_q(3, 0, "spare"))
        for m in range(CT):
            filler.append(lambda m=m: proj_q(m, 1, "spare"))

        # ---------------- stage D (emitted later, per n) --------------------
        xf32 = [pc.tile([128, L], F32, tag=f"xf{t}", name=f"xf{t}")
                for t in range(CT)]

        def emit_xf32():
            for t in range(CT):
                nc.gpsimd.tensor_copy(xf32[t][:, :], x_sb[t][:, :])
        ybig = pc.tile([128, 4 * L], F32, tag="ybig")
        ysq_t = [pc.tile([128, 512], F32R, tag=f"ysq{m}", name=f"ysq{m}")
                 for m in range(CT)]

        def stage_d(n):
            ns = slice(n * 512, (n + 1) * 512)
            ops = []
            for m in range(CT):
                def dproj(m=m):
                    ps = psum.tile([128, 512], F32, tag="spare",
                                   name=f"yp{m}{n}", bufs=2)
                    for t in range(CT):
                        nc.tensor.matmul(
                            out=ps[:, :],
                            lhsT=wo_sb[t][:, m * 128:(m + 1) * 128],
                            rhs=ao_sb[t][:, ns],
                            start=(t == 0), stop=(t == CT - 1))
                    ysl = ybig[:, m * L + n * 512: m * L + (n + 1) * 512]
                    nc.vector.tensor_scalar_add(ysl, ps[:, :], bo_sb[m])
                    eng = nc.gpsimd if m % 2 == 0 else nc.vector
                    eng.tensor_mul(ysq_t[m][:, :], ysl, ysl)
                ops.append(dproj)

            def dstat():
                ssy = psum.tile([128, 512], F32, tag="sim", bufs=2,
                                name=f"ssy{n}")
                for c in range(4):
                    for m in range(CT):
                        nc.tensor.matmul(
                            out=ssy[:, 2 * c:2 * c + 2],
                            lhsT=ysq_t[m][:, c * 128:(c + 1) * 128],
                            rhs=ones_sb[:, 0:2],
                            start=(m == 0), stop=(m == CT - 1))
                ry = pc.tile([128, 8], F32, tag=f"ry{n}")
                dve_rsqrt(ry, ssy, 8, C, f"ry{n}")
                bcy = psum.tile([128, 512], F32, tag="spare", name=f"bcy{n}",
                                bufs=2)
                for c in range(4):
                    dg = pc.tile([128, 128], F32R, tag="diag",
                                 name=f"dgy{n}{c}", bufs=4)
                    engd = nc.gpsimd if c % 2 == 0 else nc.vector
                    engd.tensor_scalar_mul(dg[:, :],
                                           ident_sb[:, :].bitcast(F32),
                                           ry[:, 2 * c:2 * c + 1])
                    nc.tensor.matmul(out=bcy[:, c * 128:(c + 1) * 128],
                                     lhsT=ones_sb[:, :], rhs=dg[:, :],
                                     start=True, stop=True)
                for m in range(CT):
                    ysl = ybig[:, m * L + n * 512: m * L + (n + 1) * 512]
                    tmp = pc.tile([128, 512], F32, tag="fintmp",
                                  name=f"ft{n}{m}", bufs=2)
                    nc.vector.scalar_tensor_tensor(
                        out=tmp[:, :], in0=ysl, scalar=g2_sb[m],
                        in1=bcy[:, :], op0=OP.mult, op1=OP.mult)
                    enga = nc.gpsimd if m % 2 == 0 else nc.vector
                    enga.tensor_add(ysl, tmp[:, :], xf32[m][:, ns])
                    nc.sync.dma_start(
                        out=y_d[m * 128:(m + 1) * 128, ns], in_=ysl)
            ops.append(dstat)
            return ops

        # ---------------- stage C: attention -------------------------------
        pexp = top.enter_context(tc.tile_pool(name="exp", bufs=1))

        steps = [(n, p, j) for n in range(2) for p in range(4)
                 for j in range(JT)]

        sim_slots = {}
        ex_slots = {}

        def emit_sims(step):
            n, p, j = step
            ns = slice(n * 512, (n + 1) * 512)
            js = slice(j * 128, (j + 1) * 128)
            sl = psum.tile([128, 1024], F32, tag="sim", bufs=2,
                           name=f"sim{n}{p}{j}")
            nc.tensor.matmul(out=sl[:, 0:512],
                             lhsT=k_sb[p][0:64, js],
                             rhs=q_sb[p][0:64, ns],
                             start=True, stop=True)
            nc.tensor.matmul(out=sl[:, 512:1024],
                             lhsT=k_sb[p][64:128, js],
                             rhs=q_sb[p][64:128, ns],
                             start=True, stop=True)
            sim_slots[step] = sl

        def emit_exps(step):
            n, p, j = step
            ex = pexp.tile([128, 1024], F32R, tag="ex", bufs=4,
                           name=f"ex{n}{p}{j}")
            nc.scalar.activation(out=ex[:, :], in_=sim_slots[step][:, :],
                                 func=AF.Exp, bias=0.0,
                                 scale=rsq_xc[:, 16 + 2 * j:17 + 2 * j])
            ex_slots[step] = ex

        ou_cur = {}

        def emit_pv(step):
            n, p, j = step
            if j == 0:
                ou_cur[0] = psum.tile([128, 512], F32, tag="ou", bufs=2,
                                      name=f"ou{n}{p}0")
                ou_cur[1] = psum.tile([128, 512], F32, tag="ou", bufs=2,
                                      name=f"ou{n}{p}1")
            ex = ex_slots[step]
            for hi in range(2):
                h = 2 * p + hi  # global head
                nc.tensor.matmul(
                    out=ou_cur[hi][0:VW, :],
                    lhsT=vT_sb[j][:, h * VW:(h + 1) * VW],
                    rhs=ex[:, hi * 512:(hi + 1) * 512],
                    start=(j == 0), stop=(j == JT - 1))

        def emit_pair_end1(step):
            # copy ou (incl. denominator row) to SBUF; frees the ou banks
            n, p, j = step
            osbs = []
            for hi in range(2):
                osb = pc.tile([VW, 512], F32R, tag="ousb",
                              name=f"osb{n}{p}{hi}", bufs=4)
                nc.vector.tensor_copy(osb[:, :], ou_cur[hi][0:VW, :])
                osbs.append(osb)
            return osbs

        def emit_pair_end2(step, osbs):
            # broadcast raw denominators, approx-reciprocal, normalize
            n, p, j = step
            ns = slice(n * 512, (n + 1) * 512)
            for hi in range(2):
                osb = osbs[hi]
                bcr = psum.tile([128, 512], F32, tag="spare", bufs=2,
                                name=f"bcr{n}{p}{hi}")
                nc.tensor.matmul(out=bcr[0:HD, :],
                                 lhsT=ones_sb[64:65, 0:HD],
                                 rhs=osb[HD:VW, :],
                                 start=True, stop=True)
                rbc = pc.tile([64, 512], F32, tag="rbc",
                              name=f"rbc{n}{p}{hi}", bufs=2)
                nc.vector.reciprocal_approx_fast(out=rbc[:, :],
                                                 in_=bcr[0:HD, :])
                nc.gpsimd.tensor_mul(
                    ao_sb[p][hi * HD:(hi + 1) * HD, ns],
                    osb[0:HD, :].bitcast(F32), rbc[:, :])

        # ---- emission with software pipelining ----
        d_ops = []
        pend2 = None
        emit_sims(steps[0])
        for si, step in enumerate(steps):
            n, p, j = step
            if si == 2:
                emit_xf32()
            emit_exps(step)
            if si + 1 < len(steps):
                emit_sims(steps[si + 1])
            emit_pv(step)
            if pend2 is not None:
                emit_pair_end2(*pend2)
                pend2 = None
            if j == JT - 1:
                osbs = emit_pair_end1(step)
                pend2 = (step, osbs)
                if (n, p) == (0, 3):
                    d_ops = stage_d(0)
            # drain deferred work into PE slack: one PSUM-serial group
            # every other step so the PE FIFO never stalls on a bank WAR
            if si % 2 == 1:
                if filler:
                    filler.pop(0)()
                elif d_ops and si >= 34:
                    d_ops.pop(0)()
        if pend2 is not None:
            emit_pair_end2(*pend2)
        for op in d_ops:
            op()
        for op in stage_d(1):
            op()

    nc.compile()
    return nc


_NC_CACHE = {}


def _get_nc():
    if "nc" not in _NC_CACHE:
        _NC_CACHE["nc"] = build()
    return _NC_CACHE["nc"]


def kernel(x, context, Wq, Wkv, Wo, bo, g, g2):
    x = np.asarray(x, dtype=np.float32)
    context = np.asarray(context, dtype=np.float32)
    Wq = np.asarray(Wq, dtype=np.float32)
    Wkv = np.asarray(Wkv, dtype=np.float32)
    Wo = np.asarray(Wo, dtype=np.float32)
    bo = np.asarray(bo, dtype=np.float32)
    g = np.asarray(g, dtype=np.float32)
    g2 = np.asarray(g2, dtype=np.float32)

    bf = ml_dtypes.bfloat16
    scale = HD ** -0.5
    wq_h = np.ascontiguousarray((Wq * g[None, :] * scale).T).astype(bf)
    wk_h = np.ascontiguousarray((Wkv[:HID] * g[None, :]).T).astype(bf)
    wv_h = np.ascontiguousarray((Wkv[HID:] * g[None, :]).T).astype(bf)
    wo_h = np.ascontiguousarray(Wo.T).astype(bf)
    bog2T = np.ascontiguousarray(np.stack([bo, g2], axis=0))  # [2, C]
    ones = np.ones((128, 128), dtype=np.float32)
    ident = np.eye(128, dtype=np.float32)

    nc = _get_nc()
    global _last_in_maps
    in_maps = []
    for i in range(NCORES):
        in_maps.append({
            "x": np.ascontiguousarray(x[i].reshape(C, L)).astype(bf),
            "ctxT": np.ascontiguousarray(context[i].T).astype(bf),
            "wq": wq_h, "wk": wk_h, "wv": wv_h, "wo": wo_h,
            "ones": ones, "ident": ident, "bog2T": bog2T,
        })
    _last_in_maps = in_maps
    res = run_bass_kernel_spmd(nc, in_maps, list(range(NCORES)))
    out = np.stack([res.results[i]["y_out"].reshape(C, H, W)
                    for i in range(NCORES)])
    return out.astype(np.float32)


_last_in_maps = None


# revision 29
# speedup vs baseline: 1.3187x; 1.3187x over previous
"""Trainium2 Bass kernel for nn_CrossAttention (B=8, C=512, H=W=32, Lc=1024,
8 heads x 64 dim).

Sharding: data-parallel over batch B across the 8 NeuronCores (1 image/core,
no collectives). Feature-on-partitions layout; all matmuls contract over SBUF
partitions.

Optimizations over the v1 kernel (271.8us):
  - bf16 inputs (host-cast): 4MB instead of 8MB HBM per core.
  - Input DMAs spread across both HWDGE rings (sync + scalar) + SWDGE; no
    4-byte-element descriptor DMAs.
  - sim matmuls row-tiled: the two K=64 heads of a pair run concurrently in
    separate PE row-groups (col tiling is not supported by the compiler).
  - PV uses the ones-augmented vT (65 cols/head) so softmax denominators
    fall out of the PV matmul for free.
  - ACT does ONLY Exp (one table load). RMS rsqrt rows are computed in
    transposed [128, n] form via per-chunk stats matmuls (N=2), then a
    Quake-seed + 2x Newton rsqrt on DVE. Softmax reciprocal via
    nc.vector.reciprocal.
  - Per-token RMS factor rc of the context is applied for free as the
    per-partition ACT scale of the exp, and folded into vT at copy time.
  - exp runs over [128, 1024] PSUM pair-tiles (two heads per ACT op).
  - PE warmed up with dummy matmuls during the input-DMA window (HAM clock).
  - n-major stage C; stage D of n=0 hides under the ACT window of n=1.
  - software-pipelined emission: per-engine FIFOs never stall on bank WARs.
"""

import numpy as np
import ml_dtypes
from contextlib import ExitStack

import concourse.bass as bass
from concourse import bacc
import concourse.mybir as mybir
import concourse.tile as tile
from concourse.bass_utils import run_bass_kernel_spmd

F32 = mybir.dt.float32
F32R = mybir.dt.float32r
BF16 = mybir.dt.bfloat16
I32 = mybir.dt.int32
AF = mybir.ActivationFunctionType
OP = mybir.AluOpType

B, C, H, W = 8, 512, 32, 32
L = H * W  # 1024 query pixels
LC = 1024  # context tokens
HEADS, HD = 8, 64
VW = HD + 1  # 65: v columns + ones column (emits softmax denominator)
HID = HEADS * HD  # 512
EPS = 1e-6
NCORES = 8
CT = C // 128  # 4 c-tiles
JT = LC // 128  # 8 j-tiles

MAGIC = 0x5F3759DF


def build():
    nc = bacc.Bacc("TRN2", target_bir_lowering=False, debug=False,
                   num_devices=NCORES)

    x_d = nc.dram_tensor("x", [C, L], BF16, kind="ExternalInput")
    ct_d = nc.dram_tensor("ctxT", [C, LC], BF16, kind="ExternalInput")
    wq_d = nc.dram_tensor("wq", [C, HID], BF16, kind="ExternalInput")
    wk_d = nc.dram_tensor("wk", [C, HID], BF16, kind="ExternalInput")
    wv_d = nc.dram_tensor("wv", [C, HID], BF16, kind="ExternalInput")
    wo_d = nc.dram_tensor("wo", [HID, C], BF16, kind="ExternalInput")
    ones_d = nc.dram_tensor("ones", [128, 128], F32R, kind="ExternalInput")
    ident_d = nc.dram_tensor("ident", [128, 128], F32R, kind="ExternalInput")
    bog2_d = nc.dram_tensor("bog2T", [2, C], F32R, kind="ExternalInput")
    y_d = nc.dram_tensor("y_out", [C, L], F32, kind="ExternalOutput")

    with tile.TileContext(nc) as tc, ExitStack() as top:
        pc = top.enter_context(tc.tile_pool(name="main", bufs=1))
        psum = top.enter_context(tc.tile_pool(name="ps", bufs=1, space="PSUM"))

        # ---------------- input DMAs (spread across rings) ----------------
        # sync ring: x then ctx (nothing else ever runs on sync).
        # gpsimd ring: consts + weights (the ACT queue stays empty so the
        # x^2 squares and exps can start the moment data lands).
        ct_sb, x_sb = [], []
        for t in range(CT):
            xt = pc.tile([128, L], BF16, tag=f"x{t}")
            nc.sync.dma_start(out=xt, in_=x_d[t * 128:(t + 1) * 128, :])
            x_sb.append(xt)
        for t in range(CT):
            ctt = pc.tile([128, LC], BF16, tag=f"ct{t}")
            nc.sync.dma_start(out=ctt, in_=ct_d[t * 128:(t + 1) * 128, :])
            ct_sb.append(ctt)
        ones_sb = pc.tile([128, 128], F32R, tag="ones")
        nc.gpsimd.dma_start(out=ones_sb, in_=ones_d[:, :])
        ident_sb = pc.tile([128, 128], F32R, tag="ident")
        nc.gpsimd.dma_start(out=ident_sb, in_=ident_d[:, :])
        wk_sb, wq_sb, wv_sb = [], [], []
        for name, lst, dram in (("wk", wk_sb, wk_d), ("wq", wq_sb, wq_d),
                                ("wv", wv_sb, wv_d)):
            for t in range(CT):
                wt = pc.tile([128, HID], BF16, tag=f"{name}{t}")
                nc.gpsimd.dma_start(out=wt, in_=dram[t * 128:(t + 1) * 128, :])
                lst.append(wt)
        bog2_sb = pc.tile([2, C], F32R, tag="bog2")
        nc.gpsimd.dma_start(out=bog2_sb, in_=bog2_d[:, :])
        wo_sb = []
        for t in range(CT):
            wt = pc.tile([128, C], BF16, tag=f"wo{t}")
            nc.gpsimd.dma_start(out=wt, in_=wo_d[t * 128:(t + 1) * 128, :])
            wo_sb.append(wt)

        # ---------------- PE warmup (runs during DMA wait) ----------------
        warm_sb = pc.tile([128, 512], F32, tag="warm")
        nc.vector.memset(warm_sb, 1.0)
        warm_ps = psum.tile([128, 512], F32, tag="spare", name="warmps",
                            bufs=2)
        for i in range(10):
            nc.tensor.matmul(out=warm_ps[:, :],
                             lhsT=warm_sb[:, 0:128].bitcast(F32R),
                             rhs=warm_sb[:, :].bitcast(F32R),
                             start=True, stop=True)
        warm_ex = pc.tile([1, 8], F32R, tag="warmex")
        nc.scalar.activation(out=warm_ex[:, :], in_=warm_sb[0:1, 0:8],
                             func=AF.Exp, bias=0.0, scale=0.0)

        # ---------------- squares + transposed stats -----------------------
        # ssq cols (pairs, col 2c used): 0:16 x-pixel chunks, 16:32 ctx
        ssq_ps = psum.tile([128, 512], F32, tag="sim", name="ssqps", bufs=2)
        sq_x, sq_c = [], []
        for t in range(CT):
            s = pc.tile([128, L], F32R, tag="sq", name=f"sqx{t}", bufs=8)
            nc.scalar.activation(out=s[:, :], in_=x_sb[t][:, :],
                                 func=AF.Square, bias=0.0, scale=1.0)
            sq_x.append(s)
        for t in range(CT):
            s = pc.tile([128, LC], F32R, tag="sq", name=f"sqc{t}", bufs=8)
            nc.vector.tensor_mul(s[:, :], ct_sb[t][:, :], ct_sb[t][:, :])
            sq_c.append(s)
        for c in range(8):
            for t in range(CT):
                nc.tensor.matmul(out=ssq_ps[:, 2 * c:2 * c + 2],
                                 lhsT=sq_x[t][:, c * 128:(c + 1) * 128],
                                 rhs=ones_sb[:, 0:2],
                                 start=(t == 0), stop=(t == CT - 1))
        for c in range(8):
            for t in range(CT):
                nc.tensor.matmul(out=ssq_ps[:, 16 + 2 * c:18 + 2 * c],
                                 lhsT=sq_c[t][:, c * 128:(c + 1) * 128],
                                 rhs=ones_sb[:, 0:2],
                                 start=(t == 0), stop=(t == CT - 1))

        # Quake rsqrt on DVE: dst = (src/nfeat + eps)^-0.5
        kmagic = pc.tile([128, 32], I32, tag="kmagic")
        nc.vector.memset(kmagic, MAGIC)

        def dve_rsqrt(dst, src_ps, ncols, nfeat, scratch_tag):
            m = pc.tile([128, ncols], F32, tag=f"{scratch_tag}m")
            nc.vector.tensor_scalar(out=m[:, :], in0=src_ps[:, 0:ncols],
                                    scalar1=1.0 / nfeat, scalar2=EPS,
                                    op0=OP.mult, op1=OP.add)
            m2 = pc.tile([128, ncols], F32, tag=f"{scratch_tag}m2")
            nc.vector.tensor_scalar(out=m2[:, :], in0=src_ps[:, 0:ncols],
                                    scalar1=0.5 / nfeat, scalar2=0.5 * EPS,
                                    op0=OP.mult, op1=OP.add)
            sh = pc.tile([128, ncols], I32, tag=f"{scratch_tag}sh")
            nc.vector.tensor_scalar(out=sh[:, :],
                                    in0=m[:, :].bitcast(I32),
                                    scalar1=1, scalar2=0,
                                    op0=OP.logical_shift_right,
                                    op1=OP.logical_shift_right)
            y0 = pc.tile([128, ncols], F32, tag=f"{scratch_tag}y0")
            nc.vector.scalar_tensor_tensor(
                out=y0[:, :].bitcast(I32), in0=kmagic[:, 0:ncols], scalar=0,
                in1=sh[:, :], op0=OP.add, op1=OP.subtract)
            # 2 Newton iters, negated form (signs cancel):
            # y' = (m2*y^2 - 1.5) * y
            t1 = pc.tile([128, ncols], F32, tag=f"{scratch_tag}t1")
            y1 = pc.tile([128, ncols], F32, tag=f"{scratch_tag}y1")
            nc.vector.tensor_mul(t1[:, :], y0[:, :], y0[:, :])
            nc.vector.tensor_mul(t1[:, :], t1[:, :], m2[:, :])
            nc.vector.scalar_tensor_tensor(
                out=y1[:, :], in0=t1[:, :], scalar=1.5, in1=y0[:, :],
                op0=OP.subtract, op1=OP.mult)
            nc.vector.tensor_mul(t1[:, :], y1[:, :], y1[:, :])
            nc.vector.tensor_mul(t1[:, :], t1[:, :], m2[:, :])
            nc.vector.scalar_tensor_tensor(
                out=dst[:, :], in0=t1[:, :], scalar=1.5, in1=y1[:, :],
                op0=OP.subtract, op1=OP.mult)

        # ---------------- projection machinery -----------------------------
        q_sb = [pc.tile([128, L], F32R, tag=f"q{m}", name=f"q{m}")
                for m in range(CT)]
        k_sb = [pc.tile([128, LC], F32R, tag=f"k{m}", name=f"k{m}")
                for m in range(CT)]
        vT_sb = []
        for j in range(JT):
            vt = pc.tile([128, HEADS * VW], F32R, tag=f"vT{j}", name=f"vT{j}")
            vh = vt[:, :].rearrange("p (h c) -> p h c", h=HEADS)
            nc.vector.memset(vh[:, :, HD:VW].bitcast(F32), 1.0)
            vT_sb.append(vt)
        ao_sb = [pc.tile([128, L], BF16, tag=f"ao{m}", name=f"ao{m}")
                 for m in range(CT)]
        rsq_xc = pc.tile([128, 32], F32, tag="rsqxc")
        bcx_sb = pc.tile([128, L], F32R, tag="bcx")

        def proj_q(m, n, ptag):
            ns = slice(n * 512, (n + 1) * 512)
            ps = psum.tile([128, 512], F32, tag=ptag, name=f"qp{m}{n}",
                           bufs=2)
            for t in range(CT):
                nc.tensor.matmul(out=ps[:, :],
                                 lhsT=wq_sb[t][:, m * 128:(m + 1) * 128],
                                 rhs=x_sb[t][:, ns],
                                 start=(t == 0), stop=(t == CT - 1))
            nc.vector.tensor_mul(q_sb[m][:, ns], ps[:, :],
                                 bcx_sb[:, ns].bitcast(F32))

        def proj_k(m, h, ptag):
            hs = slice(h * 512, (h + 1) * 512)
            ps = psum.tile([128, 512], F32, tag=ptag, name=f"kp{m}{h}",
                           bufs=2)
            for t in range(CT):
                nc.tensor.matmul(out=ps[:, :],
                                 lhsT=wk_sb[t][:, m * 128:(m + 1) * 128],
                                 rhs=ct_sb[t][:, hs],
                                 start=(t == 0), stop=(t == CT - 1))
            nc.vector.tensor_copy(k_sb[m][:, hs], ps[:, :])

        def proj_v(j, ptag):
            ps = psum.tile([128, HID], F32, tag=ptag, name=f"vp{j}",
                           bufs=2)
            for t in range(CT):
                nc.tensor.matmul(out=ps[:, :],
                                 lhsT=ct_sb[t][:, j * 128:(j + 1) * 128],
                                 rhs=wv_sb[t][:, :],
                                 start=(t == 0), stop=(t == CT - 1))
            vh = vT_sb[j][:, :].rearrange("p (h c) -> p h c", h=HEADS)
            # fold per-token rms factor rc into v
            nc.vector.tensor_scalar_mul(
                vh[:, :, 0:HD],
                ps[:, :].rearrange("p (h c) -> p h c", h=HEADS),
                rsq_xc[:, 16 + 2 * j:17 + 2 * j])

        # k projections first on the PE queue (only need ctx + wk DMAs)
        proj_k(0, 0, "spare")
        proj_k(0, 1, "ou")
        proj_k(1, 0, "spare")
        proj_k(1, 1, "ou")

        # rsq_xc cols (2c): 0:16 pixels rxT, 16:32 tokens rcT.
        # Two chains so the sim-gating rx path starts without waiting for
        # the ctx stats.
        dve_rsqrt(rsq_xc[:, 0:16], ssq_ps[:, 0:16], 16, C, "rsx")
        dve_rsqrt(rsq_xc[:, 16:32], ssq_ps[:, 16:32], 16, C, "rsc")

        # bc_rx [128, L]: bc_rx[p, i] = rx[i] via diag trick
        diag_t = [pc.tile([128, 128], F32R, tag="diag", name=f"dg{c}", bufs=4)
                  for c in range(8)]
        bcx_ps = psum.tile([128, L], F32, tag="sim", name="bcxps", bufs=2)
        for c in range(8):
            nc.vector.tensor_scalar_mul(diag_t[c][:, :],
                                        ident_sb[:, :].bitcast(F32),
                                        rsq_xc[:, 2 * c:2 * c + 1])
            nc.tensor.matmul(out=bcx_ps[:, c * 128:(c + 1) * 128],
                             lhsT=ones_sb[:, :], rhs=diag_t[c][:, :],
                             start=True, stop=True)
        nc.vector.tensor_copy(bcx_sb[:, :], bcx_ps[:, :])

        # bog2 "transpose": [2, C] row layout -> [128, 2] per c-tile
        bo_sb, g2_sb = [], []
        for t in range(CT):
            bps = psum.tile([128, 512], F32, tag="ou", name=f"bog{t}", bufs=2)
            nc.tensor.matmul(out=bps[:, 0:2],
                             lhsT=bog2_sb[:, t * 128:(t + 1) * 128],
                             rhs=ident_sb[0:2, 0:2],
                             start=True, stop=True)
            bg = pc.tile([128, 2], F32, tag=f"bog2s{t}")
            nc.vector.tensor_copy(bg[:, :], bps[:, 0:2])
            bo_sb.append(bg[:, 0:1])
            g2_sb.append(bg[:, 1:2])

        # rest of the pre-attention projections
        proj_q(0, 0, "spare")
        proj_q(1, 0, "ou")
        proj_v(0, "spare")
        proj_v(1, "ou")
        proj_v(2, "spare")
        proj_v(3, "ou")
        proj_v(4, "spare")
        proj_v(5, "ou")

        # deferred projection work, drained into stage-C PE slack
        filler = []
        filler.append(lambda: proj_v(6, "spare"))
        filler.append(lambda: proj_v(7, "spare"))
        filler.append(lambda: proj_k(2, 0, "spare"))
        filler.append(lambda: proj_k(2, 1, "spare"))
        filler.append(lambda: proj_k(3, 0, "spare"))
        filler.append(lambda: proj_k(3, 1, "spare"))
        filler.append(lambda: proj_q(2, 0, "spare"))
        filler.append(lambda: proj_q(3, 0, "spare"))
        for m in range(CT):
            filler.append(lambda m=m: proj_q(m, 1, "spare"))

        # ---------------- stage D (emitted later, per n) --------------------
        xf32 = [pc.tile([128, L], F32, tag=f"xf{t}", name=f"xf{t}")
                for t in range(CT)]

        def emit_xf32(t):
            nc.gpsimd.tensor_copy(xf32[t][:, :], x_sb[t][:, :])
        ybig = pc.tile([128, 4 * L], F32, tag="ybig")
        ysq_t = [pc.tile([128, 512], F32R, tag=f"ysq{m}", name=f"ysq{m}")
                 for m in range(CT)]

        def stage_d(n):
            ns = slice(n * 512, (n + 1) * 512)
            ops = []
            for m in range(CT):
                def dproj(m=m):
                    ps = psum.tile([128, 512], F32, tag="spare",
                                   name=f"yp{m}{n}", bufs=2)
                    for t in range(CT):
                        nc.tensor.matmul(
                            out=ps[:, :],
                            lhsT=wo_sb[t][:, m * 128:(m + 1) * 128],
                            rhs=ao_sb[t][:, ns],
                            start=(t == 0), stop=(t == CT - 1))
                    ysl = ybig[:, m * L + n * 512: m * L + (n + 1) * 512]
                    nc.vector.tensor_scalar_add(ysl, ps[:, :], bo_sb[m])
                    eng = nc.gpsimd if m % 2 == 0 else nc.vector
                    eng.tensor_mul(ysq_t[m][:, :], ysl, ysl)
                ops.append(dproj)

            def dstat():
                ssy = psum.tile([128, 512], F32, tag="sim", bufs=2,
                                name=f"ssy{n}")
                for c in range(4):
                    for m in range(CT):
                        nc.tensor.matmul(
                            out=ssy[:, 2 * c:2 * c + 2],
                            lhsT=ysq_t[m][:, c * 128:(c + 1) * 128],
                            rhs=ones_sb[:, 0:2],
                            start=(m == 0), stop=(m == CT - 1))
                ry = pc.tile([128, 8], F32, tag=f"ry{n}")
                dve_rsqrt(ry, ssy, 8, C, f"ry{n}")
                bcy = psum.tile([128, 512], F32, tag="spare", name=f"bcy{n}",
                                bufs=2)
                for c in range(4):
                    dg = pc.tile([128, 128], F32R, tag="diag",
                                 name=f"dgy{n}{c}", bufs=4)
                    engd = nc.gpsimd if c % 2 == 0 else nc.vector
                    engd.tensor_scalar_mul(dg[:, :],
                                           ident_sb[:, :].bitcast(F32),
                                           ry[:, 2 * c:2 * c + 1])
                    nc.tensor.matmul(out=bcy[:, c * 128:(c + 1) * 128],
                                     lhsT=ones_sb[:, :], rhs=dg[:, :],
                                     start=True, stop=True)
                for m in range(CT):
                    ysl = ybig[:, m * L + n * 512: m * L + (n + 1) * 512]
                    tmp = pc.tile([128, 512], F32, tag="fintmp",
                                  name=f"ft{n}{m}", bufs=2)
                    nc.vector.scalar_tensor_tensor(
                        out=tmp[:, :], in0=ysl, scalar=g2_sb[m],
                        in1=bcy[:, :], op0=OP.mult, op1=OP.mult)
                    enga = nc.gpsimd if m % 2 == 0 else nc.vector
                    enga.tensor_add(ysl, tmp[:, :], xf32[m][:, ns])
                    nc.sync.dma_start(
                        out=y_d[m * 128:(m + 1) * 128, ns], in_=ysl)
            ops.append(dstat)
            return ops

        # ---------------- stage C: attention -------------------------------
        pexp = top.enter_context(tc.tile_pool(name="exp", bufs=1))

        steps = [(n, p, j) for n in range(2) for p in range(4)
                 for j in range(JT)]

        sim_slots = {}
        ex_slots = {}

        def emit_sims(step):
            n, p, j = step
            ns = slice(n * 512, (n + 1) * 512)
            js = slice(j * 128, (j + 1) * 128)
            sl = psum.tile([128, 1024], F32, tag="sim", bufs=2,
                           name=f"sim{n}{p}{j}")
            nc.tensor.matmul(out=sl[:, 0:512],
                             lhsT=k_sb[p][0:64, js],
                             rhs=q_sb[p][0:64, ns],
                             start=True, stop=True)
            nc.tensor.matmul(out=sl[:, 512:1024],
                             lhsT=k_sb[p][64:128, js],
                             rhs=q_sb[p][64:128, ns],
                             start=True, stop=True)
            sim_slots[step] = sl

        def emit_exps(step):
            n, p, j = step
            ex = pexp.tile([128, 1024], F32R, tag="ex", bufs=4,
                           name=f"ex{n}{p}{j}")
            nc.scalar.activation(out=ex[:, :], in_=sim_slots[step][:, :],
                                 func=AF.Exp, bias=0.0,
                                 scale=rsq_xc[:, 16 + 2 * j:17 + 2 * j])
            ex_slots[step] = ex

        ou_cur = {}

        def emit_pv(step):
            n, p, j = step
            if j == 0:
                ou_cur[0] = psum.tile([128, 512], F32, tag="ou", bufs=2,
                                      name=f"ou{n}{p}0")
                ou_cur[1] = psum.tile([128, 512], F32, tag="ou", bufs=2,
                                      name=f"ou{n}{p}1")
            ex = ex_slots[step]
            for hi in range(2):
                h = 2 * p + hi  # global head
                nc.tensor.matmul(
                    out=ou_cur[hi][0:VW, :],
                    lhsT=vT_sb[j][:, h * VW:(h + 1) * VW],
                    rhs=ex[:, hi * 512:(hi + 1) * 512],
                    start=(j == 0), stop=(j == JT - 1))

        def emit_pair_end1(step):
            # copy ou (incl. denominator row) to SBUF; frees the ou banks
            n, p, j = step
            osbs = []
            for hi in range(2):
                osb = pc.tile([VW, 512], F32R, tag="ousb",
                              name=f"osb{n}{p}{hi}", bufs=4)
                nc.vector.tensor_copy(osb[:, :], ou_cur[hi][0:VW, :])
                osbs.append(osb)
            return osbs

        def emit_pair_end2(step, osbs):
            # broadcast raw denominators, approx-reciprocal, normalize
            n, p, j = step
            ns = slice(n * 512, (n + 1) * 512)
            for hi in range(2):
                osb = osbs[hi]
                bcr = psum.tile([128, 512], F32, tag="ou", bufs=2,
                                name=f"bcr{n}{p}{hi}")
                nc.tensor.matmul(out=bcr[0:HD, :],
                                 lhsT=ones_sb[64:65, 0:HD],
                                 rhs=osb[HD:VW, :],
                                 start=True, stop=True)
                rbc = pc.tile([64, 512], F32, tag="rbc",
                              name=f"rbc{n}{p}{hi}", bufs=2)
                nc.vector.reciprocal_approx_fast(out=rbc[:, :],
                                                 in_=bcr[0:HD, :])
                nc.gpsimd.tensor_mul(
                    ao_sb[p][hi * HD:(hi + 1) * HD, ns],
                    osb[0:HD, :].bitcast(F32), rbc[:, :])

        # ---- emission with software pipelining ----
        d_ops = []
        emit_sims(steps[0])
        for si, step in enumerate(steps):
            n, p, j = step
            if si in (6, 14, 22, 30):
                emit_xf32((si - 6) // 8)
            emit_exps(step)
            if si + 1 < len(steps):
                emit_sims(steps[si + 1])
            emit_pv(step)
            if j == JT - 1:
                osbs = emit_pair_end1(step)
                emit_pair_end2(step, osbs)
                if (n, p) == (0, 3):
                    d_ops = stage_d(0)
            # drain deferred work into PE slack: one PSUM-serial group
            # every other step so the PE FIFO never stalls on a bank WAR
            if si % 2 == 1:
                if filler:
                    filler.pop(0)()
                elif d_ops and si >= 34:
                    d_ops.pop(0)()
        for op in d_ops:
            op()
        for op in stage_d(1):
            op()

    nc.compile()
    return nc


_NC_CACHE = {}


def _get_nc():
    if "nc" not in _NC_CACHE:
        _NC_CACHE["nc"] = build()
    return _NC_CACHE["nc"]


def kernel(x, context, Wq, Wkv, Wo, bo, g, g2):
    x = np.asarray(x, dtype=np.float32)
    context = np.asarray(context, dtype=np.float32)
    Wq = np.asarray(Wq, dtype=np.float32)
    Wkv = np.asarray(Wkv, dtype=np.float32)
    Wo = np.asarray(Wo, dtype=np.float32)
    bo = np.asarray(bo, dtype=np.float32)
    g = np.asarray(g, dtype=np.float32)
    g2 = np.asarray(g2, dtype=np.float32)

    bf = ml_dtypes.bfloat16
    scale = HD ** -0.5
    wq_h = np.ascontiguousarray((Wq * g[None, :] * scale).T).astype(bf)
    wk_h = np.ascontiguousarray((Wkv[:HID] * g[None, :]).T).astype(bf)
    wv_h = np.ascontiguousarray((Wkv[HID:] * g[None, :]).T).astype(bf)
    wo_h = np.ascontiguousarray(Wo.T).astype(bf)
    bog2T = np.ascontiguousarray(np.stack([bo, g2], axis=0))  # [2, C]
    ones = np.ones((128, 128), dtype=np.float32)
    ident = np.eye(128, dtype=np.float32)

    nc = _get_nc()
    global _last_in_maps
    in_maps = []
    for i in range(NCORES):
        in_maps.append({
            "x": np.ascontiguousarray(x[i].reshape(C, L)).astype(bf),
            "ctxT": np.ascontiguousarray(context[i].T).astype(bf),
            "wq": wq_h, "wk": wk_h, "wv": wv_h, "wo": wo_h,
            "ones": ones, "ident": ident, "bog2T": bog2T,
        })
    _last_in_maps = in_maps
    res = run_bass_kernel_spmd(nc, in_maps, list(range(NCORES)))
    out = np.stack([res.results[i]["y_out"].reshape(C, H, W)
                    for i in range(NCORES)])
    return out.astype(np.float32)


_last_in_maps = None


# revision 32
# speedup vs baseline: 1.3467x; 1.0212x over previous
"""Trainium2 Bass kernel for nn_CrossAttention (B=8, C=512, H=W=32, Lc=1024,
8 heads x 64 dim).

Sharding: data-parallel over batch B across the 8 NeuronCores (1 image/core,
no collectives). Feature-on-partitions layout; all matmuls contract over SBUF
partitions.

Optimizations over the v1 kernel (271.8us):
  - bf16 inputs (host-cast): 4MB instead of 8MB HBM per core.
  - Input DMAs spread across both HWDGE rings (sync + scalar) + SWDGE; no
    4-byte-element descriptor DMAs.
  - sim matmuls row-tiled: the two K=64 heads of a pair run concurrently in
    separate PE row-groups (col tiling is not supported by the compiler).
  - PV uses the ones-augmented vT (65 cols/head) so softmax denominators
    fall out of the PV matmul for free.
  - ACT does ONLY Exp (one table load). RMS rsqrt rows are computed in
    transposed [128, n] form via per-chunk stats matmuls (N=2), then a
    Quake-seed + 2x Newton rsqrt on DVE. Softmax reciprocal via
    nc.vector.reciprocal.
  - Per-token RMS factor rc of the context is applied for free as the
    per-partition ACT scale of the exp, and folded into vT at copy time.
  - exp runs over [128, 1024] PSUM pair-tiles (two heads per ACT op).
  - PE warmed up with dummy matmuls during the input-DMA window (HAM clock).
  - n-major stage C; stage D of n=0 hides under the ACT window of n=1.
  - software-pipelined emission: per-engine FIFOs never stall on bank WARs.
"""

import numpy as np
import ml_dtypes
from contextlib import ExitStack

import concourse.bass as bass
from concourse import bacc
import concourse.mybir as mybir
import concourse.tile as tile
from concourse.bass_utils import run_bass_kernel_spmd

F32 = mybir.dt.float32
F32R = mybir.dt.float32r
BF16 = mybir.dt.bfloat16
I32 = mybir.dt.int32
AF = mybir.ActivationFunctionType
OP = mybir.AluOpType

B, C, H, W = 8, 512, 32, 32
L = H * W  # 1024 query pixels
LC = 1024  # context tokens
HEADS, HD = 8, 64
VW = HD + 1  # 65: v columns + ones column (emits softmax denominator)
HID = HEADS * HD  # 512
EPS = 1e-6
NCORES = 8
CT = C // 128  # 4 c-tiles
JT = LC // 128  # 8 j-tiles

MAGIC = 0x5F3759DF


def build():
    nc = bacc.Bacc("TRN2", target_bir_lowering=False, debug=False,
                   num_devices=NCORES)

    x_d = nc.dram_tensor("x", [C, L], BF16, kind="ExternalInput")
    ct_d = nc.dram_tensor("ctxT", [C, LC], BF16, kind="ExternalInput")
    wq_d = nc.dram_tensor("wq", [C, HID], BF16, kind="ExternalInput")
    wk_d = nc.dram_tensor("wk", [C, HID], BF16, kind="ExternalInput")
    wv_d = nc.dram_tensor("wv", [C, HID], BF16, kind="ExternalInput")
    wo_d = nc.dram_tensor("wo", [HID, C], BF16, kind="ExternalInput")
    ones_d = nc.dram_tensor("ones", [128, 128], F32R, kind="ExternalInput")
    ident_d = nc.dram_tensor("ident", [128, 128], F32R, kind="ExternalInput")
    selxc_d = nc.dram_tensor("selxc", [128, 4], F32R, kind="ExternalInput")
    bog2_d = nc.dram_tensor("bog2T", [2, C], F32R, kind="ExternalInput")
    y_d = nc.dram_tensor("y_out", [C, L], F32, kind="ExternalOutput")

    with tile.TileContext(nc) as tc, ExitStack() as top:
        pc = top.enter_context(tc.tile_pool(name="main", bufs=1))
        psum = top.enter_context(tc.tile_pool(name="ps", bufs=1, space="PSUM"))

        # ---------------- input DMAs (spread across rings) ----------------
        # sync ring: x then ctx (nothing else ever runs on sync).
        # gpsimd ring: consts + weights (the ACT queue stays empty so the
        # x^2 squares and exps can start the moment data lands).
        ct_sb, x_sb = [], []
        for t in range(CT):
            xt = pc.tile([128, L], BF16, tag=f"x{t}")
            nc.sync.dma_start(out=xt, in_=x_d[t * 128:(t + 1) * 128, :])
            x_sb.append(xt)
            ctt = pc.tile([128, LC], BF16, tag=f"ct{t}")
            nc.sync.dma_start(out=ctt, in_=ct_d[t * 128:(t + 1) * 128, :])
            ct_sb.append(ctt)
        ones_sb = pc.tile([128, 128], F32R, tag="ones")
        nc.gpsimd.dma_start(out=ones_sb, in_=ones_d[:, :])
        ident_sb = pc.tile([128, 128], F32R, tag="ident")
        nc.gpsimd.dma_start(out=ident_sb, in_=ident_d[:, :])
        selxc_sb = pc.tile([128, 4], F32R, tag="selxc")
        nc.gpsimd.dma_start(out=selxc_sb, in_=selxc_d[:, :])
        wk_sb, wq_sb, wv_sb = [], [], []
        for name, lst, dram in (("wk", wk_sb, wk_d), ("wq", wq_sb, wq_d),
                                ("wv", wv_sb, wv_d)):
            for t in range(CT):
                wt = pc.tile([128, HID], BF16, tag=f"{name}{t}")
                nc.gpsimd.dma_start(out=wt, in_=dram[t * 128:(t + 1) * 128, :])
                lst.append(wt)
        bog2_sb = pc.tile([2, C], F32R, tag="bog2")
        nc.gpsimd.dma_start(out=bog2_sb, in_=bog2_d[:, :])
        wo_sb = []
        for t in range(CT):
            wt = pc.tile([128, C], BF16, tag=f"wo{t}")
            nc.gpsimd.dma_start(out=wt, in_=wo_d[t * 128:(t + 1) * 128, :])
            wo_sb.append(wt)

        # ---------------- PE warmup (runs during DMA wait) ----------------
        warm_sb = pc.tile([128, 512], F32, tag="warm")
        nc.vector.memset(warm_sb, 1.0)
        warm_ps = psum.tile([128, 512], F32, tag="spare", name="warmps",
                            bufs=2)
        for i in range(10):
            nc.tensor.matmul(out=warm_ps[:, :],
                             lhsT=warm_sb[:, 0:128].bitcast(F32R),
                             rhs=warm_sb[:, :].bitcast(F32R),
                             start=True, stop=True)
        warm_ex = pc.tile([1, 8], F32R, tag="warmex")
        nc.scalar.activation(out=warm_ex[:, :], in_=warm_sb[0:1, 0:8],
                             func=AF.Exp, bias=0.0, scale=0.0)

        # ---------------- squares + row stats + tiny transpose --------------
        # Row-form colsums (the ones column is the stationary operand, so
        # there is exactly one LDWEIGHTS for the whole stats pass), then
        # [2, 128] -> [128, 2] transposes via K=2 matmuls against the
        # identity. ssqT cols: even = x-pixel chunks, odd = ctx chunks.
        sq_x, sq_c = [], []
        for t in range(CT):
            s = pc.tile([128, L], F32R, tag="sq", name=f"sqx{t}", bufs=8)
            nc.scalar.activation(out=s[:, :], in_=x_sb[t][:, :],
                                 func=AF.Square, bias=0.0, scale=1.0)
            sq_x.append(s)
        for t in range(CT):
            s = pc.tile([128, LC], F32R, tag="sq", name=f"sqc{t}", bufs=8)
            nc.vector.tensor_mul(s[:, :], ct_sb[t][:, :], ct_sb[t][:, :])
            sq_c.append(s)
        rows2 = pc.tile([2, L], F32R, tag="rows2")
        for h in range(2):
            rp = psum.tile([128, 512], F32, tag="ou", bufs=2,
                           name=f"rows{h}")
            for t in range(CT):
                nc.tensor.matmul(out=rp[0:2, :],
                                 lhsT=selxc_sb[:, 0:2],
                                 rhs=sq_x[t][:, h * 512:(h + 1) * 512],
                                 start=(t == 0), stop=False)
            for t in range(CT):
                nc.tensor.matmul(out=rp[0:2, :],
                                 lhsT=selxc_sb[:, 2:4],
                                 rhs=sq_c[t][:, h * 512:(h + 1) * 512],
                                 start=False, stop=(t == CT - 1))
            nc.scalar.activation(out=rows2[0:2, h * 512:(h + 1) * 512],
                                 in_=rp[0:2, :], func=AF.Copy,
                                 bias=0.0, scale=1.0)
        ssq_ps = psum.tile([128, 512], F32, tag="sim", name="ssqps", bufs=2)
        for c in range(8):
            nc.tensor.matmul(out=ssq_ps[:, 2 * c:2 * c + 2],
                             lhsT=rows2[:, c * 128:(c + 1) * 128],
                             rhs=ident_sb[0:2, 0:2],
                             start=True, stop=True)

        # Quake rsqrt on DVE: dst = (src/nfeat + eps)^-0.5
        kmagic = pc.tile([128, 16], I32, tag="kmagic")
        nc.vector.memset(kmagic, MAGIC)

        def dve_rsqrt(dst, src_ps, ncols, nfeat, scratch_tag):
            m = pc.tile([128, ncols], F32, tag=f"{scratch_tag}m")
            nc.vector.tensor_scalar(out=m[:, :], in0=src_ps[:, 0:ncols],
                                    scalar1=1.0 / nfeat, scalar2=EPS,
                                    op0=OP.mult, op1=OP.add)
            m2 = pc.tile([128, ncols], F32, tag=f"{scratch_tag}m2")
            nc.vector.tensor_scalar(out=m2[:, :], in0=src_ps[:, 0:ncols],
                                    scalar1=0.5 / nfeat, scalar2=0.5 * EPS,
                                    op0=OP.mult, op1=OP.add)
            sh = pc.tile([128, ncols], I32, tag=f"{scratch_tag}sh")
            nc.vector.tensor_scalar(out=sh[:, :],
                                    in0=m[:, :].bitcast(I32),
                                    scalar1=1, scalar2=0,
                                    op0=OP.logical_shift_right,
                                    op1=OP.logical_shift_right)
            y0 = pc.tile([128, ncols], F32, tag=f"{scratch_tag}y0")
            nc.vector.scalar_tensor_tensor(
                out=y0[:, :].bitcast(I32), in0=kmagic[:, 0:ncols], scalar=0,
                in1=sh[:, :], op0=OP.add, op1=OP.subtract)
            # 2 Newton iters, negated form (signs cancel):
            # y' = (m2*y^2 - 1.5) * y
            t1 = pc.tile([128, ncols], F32, tag=f"{scratch_tag}t1")
            y1 = pc.tile([128, ncols], F32, tag=f"{scratch_tag}y1")
            nc.vector.tensor_mul(t1[:, :], y0[:, :], y0[:, :])
            nc.vector.tensor_mul(t1[:, :], t1[:, :], m2[:, :])
            nc.vector.scalar_tensor_tensor(
                out=y1[:, :], in0=t1[:, :], scalar=1.5, in1=y0[:, :],
                op0=OP.subtract, op1=OP.mult)
            nc.vector.tensor_mul(t1[:, :], y1[:, :], y1[:, :])
            nc.vector.tensor_mul(t1[:, :], t1[:, :], m2[:, :])
            nc.vector.scalar_tensor_tensor(
                out=dst[:, :], in0=t1[:, :], scalar=1.5, in1=y1[:, :],
                op0=OP.subtract, op1=OP.mult)

        # ---------------- projection machinery -----------------------------
        q_sb = [pc.tile([128, L], BF16, tag=f"q{m}", name=f"q{m}")
                for m in range(CT)]
        k_sb = [pc.tile([128, LC], BF16, tag=f"k{m}", name=f"k{m}")
                for m in range(CT)]
        vT_sb = []
        for j in range(JT):
            vt = pc.tile([128, HEADS * VW], BF16, tag=f"vT{j}", name=f"vT{j}")
            vh = vt[:, :].rearrange("p (h c) -> p h c", h=HEADS)
            nc.vector.memset(vh[:, :, HD:VW], 1.0)
            vT_sb.append(vt)
        ao_sb = [pc.tile([128, L], BF16, tag=f"ao{m}", name=f"ao{m}")
                 for m in range(CT)]
        rsq_xc = pc.tile([128, 16], F32, tag="rsqxc")
        bcx_sb = pc.tile([128, L], F32R, tag="bcx")

        def proj_q(m, n, ptag):
            ns = slice(n * 512, (n + 1) * 512)
            ps = psum.tile([128, 512], F32, tag=ptag, name=f"qp{m}{n}",
                           bufs=2)
            for t in range(CT):
                nc.tensor.matmul(out=ps[:, :],
                                 lhsT=wq_sb[t][:, m * 128:(m + 1) * 128],
                                 rhs=x_sb[t][:, ns],
                                 start=(t == 0), stop=(t == CT - 1))
            nc.vector.tensor_mul(q_sb[m][:, ns], ps[:, :],
                                 bcx_sb[:, ns].bitcast(F32))

        def proj_k(m, h, ptag):
            hs = slice(h * 512, (h + 1) * 512)
            ps = psum.tile([128, 512], F32, tag=ptag, name=f"kp{m}{h}",
                           bufs=2)
            for t in range(CT):
                nc.tensor.matmul(out=ps[:, :],
                                 lhsT=wk_sb[t][:, m * 128:(m + 1) * 128],
                                 rhs=ct_sb[t][:, hs],
                                 start=(t == 0), stop=(t == CT - 1))
            nc.vector.tensor_copy(k_sb[m][:, hs], ps[:, :])

        def proj_v(j, ptag):
            ps = psum.tile([128, HID], F32, tag=ptag, name=f"vp{j}",
                           bufs=2)
            for t in range(CT):
                nc.tensor.matmul(out=ps[:, :],
                                 lhsT=ct_sb[t][:, j * 128:(j + 1) * 128],
                                 rhs=wv_sb[t][:, :],
                                 start=(t == 0), stop=(t == CT - 1))
            vh = vT_sb[j][:, :].rearrange("p (h c) -> p h c", h=HEADS)
            # fold per-token rms factor rc into v
            nc.vector.tensor_scalar_mul(
                vh[:, :, 0:HD],
                ps[:, :].rearrange("p (h c) -> p h c", h=HEADS),
                rsq_xc[:, 2 * j + 1:2 * j + 2])

        # k projections first on the PE queue (only need ctx + wk DMAs)
        proj_k(0, 0, "spare")
        proj_k(0, 1, "ou")
        proj_k(1, 0, "spare")
        proj_k(1, 1, "ou")

        # rsq_xc cols: even = rxT pixel chunks, odd = rcT token chunks
        dve_rsqrt(rsq_xc[:, 0:16], ssq_ps[:, 0:16], 16, C, "rs")

        # bc_rx [128, L]: bc_rx[p, i] = rx[i] via diag trick
        diag_t = [pc.tile([128, 128], F32R, tag="diag", name=f"dg{c}", bufs=4)
                  for c in range(8)]
        bcx_ps = psum.tile([128, L], F32, tag="sim", name="bcxps", bufs=2)
        for c in range(8):
            nc.vector.tensor_scalar_mul(diag_t[c][:, :],
                                        ident_sb[:, :].bitcast(F32),
                                        rsq_xc[:, 2 * c:2 * c + 1])
            nc.tensor.matmul(out=bcx_ps[:, c * 128:(c + 1) * 128],
                             lhsT=ones_sb[:, :], rhs=diag_t[c][:, :],
                             start=True, stop=True)
        nc.vector.tensor_copy(bcx_sb[:, :], bcx_ps[:, :])

        # bog2 "transpose": [2, C] row layout -> [128, 2] per c-tile
        bo_sb, g2_sb = [], []
        for t in range(CT):
            bps = psum.tile([128, 512], F32, tag="ou", name=f"bog{t}", bufs=2)
            nc.tensor.matmul(out=bps[:, 0:2],
                             lhsT=bog2_sb[:, t * 128:(t + 1) * 128],
                             rhs=ident_sb[0:2, 0:2],
                             start=True, stop=True)
            bg = pc.tile([128, 2], F32, tag=f"bog2s{t}")
            nc.vector.tensor_copy(bg[:, :], bps[:, 0:2])
            bo_sb.append(bg[:, 0:1])
            g2_sb.append(bg[:, 1:2])

        # rest of the pre-attention projections
        proj_q(0, 0, "spare")
        proj_q(1, 0, "ou")
        proj_v(0, "spare")
        proj_v(1, "ou")
        proj_v(2, "spare")
        proj_v(3, "ou")
        proj_v(4, "spare")
        proj_v(5, "ou")

        # deferred projection work, drained into stage-C PE slack
        filler = []
        filler.append(lambda: proj_v(6, "spare"))
        filler.append(lambda: proj_v(7, "spare"))
        filler.append(lambda: proj_k(2, 0, "spare"))
        filler.append(lambda: proj_k(2, 1, "spare"))
        filler.append(lambda: proj_k(3, 0, "spare"))
        filler.append(lambda: proj_k(3, 1, "spare"))
        filler.append(lambda: proj_q(2, 0, "spare"))
        filler.append(lambda: proj_q(3, 0, "spare"))
        for m in range(CT):
            filler.append(lambda m=m: proj_q(m, 1, "spare"))

        # ---------------- stage D (emitted later, per n) --------------------
        xf32 = [pc.tile([128, L], F32, tag=f"xf{t}", name=f"xf{t}")
                for t in range(CT)]

        def emit_xf32(t):
            nc.gpsimd.tensor_copy(xf32[t][:, :], x_sb[t][:, :])
        ybig = pc.tile([128, 4 * L], F32, tag="ybig")
        ysq_t = [pc.tile([128, 512], F32R, tag=f"ysq{m}", name=f"ysq{m}")
                 for m in range(CT)]

        def stage_d(n):
            ns = slice(n * 512, (n + 1) * 512)
            ops = []
            for m in range(CT):
                def dproj(m=m):
                    ps = psum.tile([128, 512], F32, tag="spare",
                                   name=f"yp{m}{n}", bufs=2)
                    for t in range(CT):
                        nc.tensor.matmul(
                            out=ps[:, :],
                            lhsT=wo_sb[t][:, m * 128:(m + 1) * 128],
                            rhs=ao_sb[t][:, ns],
                            start=(t == 0), stop=(t == CT - 1))
                    ysl = ybig[:, m * L + n * 512: m * L + (n + 1) * 512]
                    nc.vector.tensor_scalar_add(ysl, ps[:, :], bo_sb[m])
                    eng = nc.gpsimd if m % 2 == 0 else nc.vector
                    eng.tensor_mul(ysq_t[m][:, :], ysl, ysl)
                ops.append(dproj)

            def dstat():
                yr = psum.tile([128, 512], F32, tag="spare", bufs=2,
                               name=f"yr{n}")
                for m in range(CT):
                    nc.tensor.matmul(out=yr[0:2, :],
                                     lhsT=selxc_sb[:, 0:2],
                                     rhs=ysq_t[m][:, :],
                                     start=(m == 0), stop=(m == CT - 1))
                rowy = pc.tile([2, 512], F32R, tag=f"rowy{n}")
                nc.scalar.activation(out=rowy[0:2, :], in_=yr[0:2, :],
                                     func=AF.Copy, bias=0.0, scale=1.0)
                ssy = psum.tile([128, 512], F32, tag="sim", bufs=2,
                                name=f"ssy{n}")
                for c in range(4):
                    nc.tensor.matmul(out=ssy[:, 2 * c:2 * c + 2],
                                     lhsT=rowy[:, c * 128:(c + 1) * 128],
                                     rhs=ident_sb[0:2, 0:2],
                                     start=True, stop=True)
                ry = pc.tile([128, 8], F32, tag=f"ry{n}")
                dve_rsqrt(ry, ssy, 8, C, f"ry{n}")
                bcy = psum.tile([128, 512], F32, tag="spare", name=f"bcy{n}",
                                bufs=2)
                for c in range(4):
                    dg = pc.tile([128, 128], F32R, tag="diag",
                                 name=f"dgy{n}{c}", bufs=4)
                    engd = nc.gpsimd if c % 2 == 0 else nc.vector
                    engd.tensor_scalar_mul(dg[:, :],
                                           ident_sb[:, :].bitcast(F32),
                                           ry[:, 2 * c:2 * c + 1])
                    nc.tensor.matmul(out=bcy[:, c * 128:(c + 1) * 128],
                                     lhsT=ones_sb[:, :], rhs=dg[:, :],
                                     start=True, stop=True)
                for m in range(CT):
                    ysl = ybig[:, m * L + n * 512: m * L + (n + 1) * 512]
                    tmp = pc.tile([128, 512], F32, tag="fintmp",
                                  name=f"ft{n}{m}", bufs=2)
                    nc.vector.scalar_tensor_tensor(
                        out=tmp[:, :], in0=ysl, scalar=g2_sb[m],
                        in1=bcy[:, :], op0=OP.mult, op1=OP.mult)
                    enga = nc.gpsimd if m % 2 == 0 else nc.vector
                    enga.tensor_add(ysl, tmp[:, :], xf32[m][:, ns])
                    nc.sync.dma_start(
                        out=y_d[m * 128:(m + 1) * 128, ns], in_=ysl)
            ops.append(dstat)
            return ops

        # ---------------- stage C: attention -------------------------------
        pexp = top.enter_context(tc.tile_pool(name="exp", bufs=1))

        steps = [(n, p, j) for n in range(2) for p in range(4)
                 for j in range(JT)]

        sim_slots = {}
        ex_slots = {}

        def emit_sims(step):
            n, p, j = step
            ns = slice(n * 512, (n + 1) * 512)
            js = slice(j * 128, (j + 1) * 128)
            sl = psum.tile([128, 1024], F32, tag="sim", bufs=2,
                           name=f"sim{n}{p}{j}")
            nc.tensor.matmul(out=sl[:, 0:512],
                             lhsT=k_sb[p][0:64, js],
                             rhs=q_sb[p][0:64, ns],
                             start=True, stop=True)
            nc.tensor.matmul(out=sl[:, 512:1024],
                             lhsT=k_sb[p][64:128, js],
                             rhs=q_sb[p][64:128, ns],
                             start=True, stop=True)
            sim_slots[step] = sl

        def emit_exps(step):
            n, p, j = step
            ex = pexp.tile([128, 1024], BF16, tag="ex", bufs=4,
                           name=f"ex{n}{p}{j}")
            nc.scalar.activation(out=ex[:, :], in_=sim_slots[step][:, :],
                                 func=AF.Exp, bias=0.0,
                                 scale=rsq_xc[:, 2 * j + 1:2 * j + 2])
            ex_slots[step] = ex

        ou_cur = {}

        def emit_pv(step):
            n, p, j = step
            if j == 0:
                ou_cur[0] = psum.tile([128, 512], F32, tag="ou", bufs=2,
                                      name=f"ou{n}{p}0")
                ou_cur[1] = psum.tile([128, 512], F32, tag="ou", bufs=2,
                                      name=f"ou{n}{p}1")
            ex = ex_slots[step]
            for hi in range(2):
                h = 2 * p + hi  # global head
                nc.tensor.matmul(
                    out=ou_cur[hi][0:VW, :],
                    lhsT=vT_sb[j][:, h * VW:(h + 1) * VW],
                    rhs=ex[:, hi * 512:(hi + 1) * 512],
                    start=(j == 0), stop=(j == JT - 1))

        def emit_pair_end1(step):
            # copy ou (incl. denominator row) to SBUF; frees the ou banks
            n, p, j = step
            osbs = []
            for hi in range(2):
                osb = pc.tile([VW, 512], F32R, tag="ousb",
                              name=f"osb{n}{p}{hi}", bufs=4)
                nc.vector.tensor_copy(osb[:, :], ou_cur[hi][0:VW, :])
                osbs.append(osb)
            return osbs

        def emit_pair_end2(step, osbs):
            # broadcast raw denominators, approx-reciprocal, normalize
            n, p, j = step
            ns = slice(n * 512, (n + 1) * 512)
            for hi in range(2):
                osb = osbs[hi]
                bcr = psum.tile([128, 512], F32, tag="spare", bufs=2,
                                name=f"bcr{n}{p}{hi}")
                nc.tensor.matmul(out=bcr[0:HD, :],
                                 lhsT=ones_sb[64:65, 0:HD],
                                 rhs=osb[HD:VW, :],
                                 start=True, stop=True)
                rbc = pc.tile([64, 512], F32, tag="rbc",
                              name=f"rbc{n}{p}{hi}", bufs=2)
                nc.vector.reciprocal_approx_fast(out=rbc[:, :],
                                                 in_=bcr[0:HD, :])
                nc.gpsimd.tensor_mul(
                    ao_sb[p][hi * HD:(hi + 1) * HD, ns],
                    osb[0:HD, :].bitcast(F32), rbc[:, :])

        # ---- emission with software pipelining ----
        d_ops = []
        pend2 = None
        emit_sims(steps[0])
        for si, step in enumerate(steps):
            n, p, j = step
            if si in (6, 14, 22, 30):
                emit_xf32((si - 6) // 8)
            emit_exps(step)
            if si + 1 < len(steps):
                emit_sims(steps[si + 1])
            emit_pv(step)
            if pend2 is not None:
                emit_pair_end2(*pend2)
                pend2 = None
            if j == JT - 1:
                osbs = emit_pair_end1(step)
                pend2 = (step, osbs)
                if (n, p) == (0, 3):
                    d_ops = stage_d(0)
            # drain deferred work into PE slack: one PSUM-serial group
            # every other step so the PE FIFO never stalls on a bank WAR
            if si % 2 == 1:
                if filler:
                    filler.pop(0)()
                elif d_ops and si >= 36:
                    d_ops.pop(0)()
        if pend2 is not None:
            emit_pair_end2(*pend2)
        for op in d_ops:
            op()
        for op in stage_d(1):
            op()

    nc.compile()
    return nc


_NC_CACHE = {}


def _get_nc():
    if "nc" not in _NC_CACHE:
        _NC_CACHE["nc"] = build()
    return _NC_CACHE["nc"]


def kernel(x, context, Wq, Wkv, Wo, bo, g, g2):
    x = np.asarray(x, dtype=np.float32)
    context = np.asarray(context, dtype=np.float32)
    Wq = np.asarray(Wq, dtype=np.float32)
    Wkv = np.asarray(Wkv, dtype=np.float32)
    Wo = np.asarray(Wo, dtype=np.float32)
    bo = np.asarray(bo, dtype=np.float32)
    g = np.asarray(g, dtype=np.float32)
    g2 = np.asarray(g2, dtype=np.float32)

    bf = ml_dtypes.bfloat16
    scale = HD ** -0.5
    wq_h = np.ascontiguousarray((Wq * g[None, :] * scale).T).astype(bf)
    wk_h = np.ascontiguousarray((Wkv[:HID] * g[None, :]).T).astype(bf)
    wv_h = np.ascontiguousarray((Wkv[HID:] * g[None, :]).T).astype(bf)
    wo_h = np.ascontiguousarray(Wo.T).astype(bf)
    bog2T = np.ascontiguousarray(np.stack([bo, g2], axis=0))  # [2, C]
    ones = np.ones((128, 128), dtype=np.float32)
    ident = np.eye(128, dtype=np.float32)
    selxc = np.zeros((128, 4), dtype=np.float32)
    selxc[:, 0] = 1.0
    selxc[:, 3] = 1.0

    nc = _get_nc()
    global _last_in_maps
    in_maps = []
    for i in range(NCORES):
        in_maps.append({
            "x": np.ascontiguousarray(x[i].reshape(C, L)).astype(bf),
            "ctxT": np.ascontiguousarray(context[i].T).astype(bf),
            "wq": wq_h, "wk": wk_h, "wv": wv_h, "wo": wo_h,
            "ones": ones, "ident": ident, "selxc": selxc,
            "bog2T": bog2T,
        })
    _last_in_maps = in_maps
    res = run_bass_kernel_spmd(nc, in_maps, list(range(NCORES)))
    out = np.stack([res.results[i]["y_out"].reshape(C, H, W)
                    for i in range(NCORES)])
    return out.astype(np.float32)


_last_in_maps = None
